# revision 8
# baseline (speedup 1.0000x reference)
"""Trainium2 Bass kernel for FeatureOnlyGate MoE routing (v2).

Math: g = h @ W.T + b  (h: [N,64], W: [6,64], b: [6])
      out[e] = [g_e >= m2] * sigmoid(2*g_e - m1 - m2)
      where m1/m2 are the top-2 logits per token (full-softmax denominator
      cancels after top-2 masking; for the top-1 entry 2g-m1-m2 = m1-m2,
      for the top-2 entry it's m2-m1, others are masked to 0).

All logit math is full fp32: min gap between 2nd and 3rd logit on this
data is 2.5e-7, so reduced-precision matmuls flip top-2 selections.

Changes vs the original baseline (302us):
  - JT=32 tokens/partition (8KB DMA lines in, 768B contiguous runs out:
    both >= 512B so no half-bandwidth DMA penalty on either direction),
    4x fewer DMA instructions, h_pool 6 deep for DMA prefetch.
  - PSUM->SBUF hT copies on ACT as 2x [128,1024] (2-bank PSUM tiles);
    logits of 2 chunks packed per PSUM bank with one fused DVE bias-add;
    g_psum 3 banks deep so the per-batch DVE top-2 burst never
    back-pressures the matmul->bias drain into the DMA stream.
  - top-2 trimmed to 8 DVE ops/batch with the sigmoid on ACT; output DMA
    issued from the Pool SWDGE queue so its semaphore wait (end of the
    DVE chain) cannot head-of-line block h-load issue on the SP queue.
  - PE work per 4096-token chunk: 16 fp32 pair-transposes + 16 12-col
    fp32 matmuls; 12 warm matmuls at start ramp the PE p-state.

Per-core dataflow (data parallel over 8 cores, 262144 tokens each):
  DMA h chunks [128 part, 2048] (32 token rows = 8KB contiguous per
  partition) -> 16x PE pair-transpose [128,128] into 4 PSUM banks ->
  ACT/Pool copy PSUM->SBUF -> 16 fp32 matmuls (stationary hT block,
  moving block-diagonal glued gate weight [128,12]) -> logits land
  token-major [128, 192] in PSUM -> Pool bias-add into batch tile ->
  batched DVE top-2 + ACT sigmoid -> contiguous DMA out.
"""

import os
import numpy as np

N_FULL = 2097152
D = 64
E = 6
NCORES = 8
NSH = N_FULL // NCORES  # 262144 tokens per core

P = 128         # partitions
JT = 32         # consecutive tokens per partition per chunk
CHUNK = P * JT  # 4096 tokens
FD = JT * E     # 192 logit elems per partition per chunk
BATCH = 4       # chunks per top-2 batch
TOKB = CHUNK * BATCH  # 16384 tokens per batch
F = BATCH * FD        # 768 logit elems per partition per batch
TB = BATCH * JT       # 128 tokens per partition per batch

WARM_N = 128    # per-chunk PE keep-hot matmul width (bf16 cols); 0=off

LAST_RESULTS = None  # BassKernelResults of the last hardware run (for test.py)


def _build_nc(nsh, repeats=1, warm=False, warm_n=WARM_N):
    import concourse.bass as bass
    from concourse import bacc
    import concourse.mybir as mybir
    from concourse.tile import TileContext

    f32 = mybir.dt.float32
    Alu = mybir.AluOpType
    Act = mybir.ActivationFunctionType

    nbatch = nsh // TOKB
    assert nsh % TOKB == 0

    nc = bacc.Bacc(None)
    h = nc.declare_dram_parameter("h", [nsh, D], f32, isOutput=False)
    ident = nc.declare_dram_parameter("ident", [P, P], f32, isOutput=False)
    wglue = nc.declare_dram_parameter("wglue", [P, 2 * E], f32, isOutput=False)
    biasr = nc.declare_dram_parameter("biasr", [P, E], f32, isOutput=False)
    out = nc.declare_dram_parameter("out", [nsh, E], f32, isOutput=True)

    # token n = chunk*4096 + 32*p + j  ->  per chunk: [128 part, 2048] with
    # 32 full token rows (8KB) contiguous per partition.
    hv = h[:, :].rearrange("(nc p j) e -> nc p (j e)", p=P, j=JT)
    # out per batch: [128 part, 4 chunk, 192]; (j x) = 768B contiguous in DRAM.
    ov = out[:, :].rearrange(
        "(nb c p j) x -> nb p c (j x)", c=BATCH, p=P, j=JT
    )

    with TileContext(nc) as tc:
        with (
            tc.tile_pool(name="const_pool", bufs=1) as const_pool,
            tc.tile_pool(name="h_pool", bufs=6) as h_pool,
            tc.tile_pool(name="hT_psum", bufs=2, space="PSUM") as hT_psum_pool,
            tc.tile_pool(name="hT_sb", bufs=3) as hT_sb_pool,
            tc.tile_pool(name="g_psum", bufs=3, space="PSUM") as g_psum_pool,
            tc.tile_pool(name="t0_pool", bufs=2) as t0_pool,
            tc.tile_pool(name="work", bufs=2) as work,
        ):
            ident_sb = const_pool.tile([P, P], f32)
            wg_sb = const_pool.tile([P, 2 * E], f32)
            bias_sb = const_pool.tile([P, E], f32)
            nc.sync.dma_start(out=ident_sb[:, :], in_=ident[:, :])
            nc.sync.dma_start(out=wg_sb[:, :], in_=wglue[:, :])
            nc.sync.dma_start(out=bias_sb[:, :], in_=biasr[:, :])

            if warm:
                bf16 = mybir.dt.bfloat16
                dummy_w = const_pool.tile([P, P], bf16)
                dummy_x = const_pool.tile([P, 512], bf16)
                nc.vector.memset(dummy_w[:, :], 0.0)
                nc.vector.memset(dummy_x[:, :], 0.0)

                warm_ps = g_psum_pool.tile([P, 512], f32, tag="warm", bufs=1)
                for _ in range(12):
                    nc.tensor.matmul(
                        warm_ps[:, :], dummy_w[:, :], dummy_x[:, :],
                        start=True, stop=True,
                    )

            import contextlib

            loop_ctx = (
                tc.For_i(0, repeats, 1)
                if repeats > 1
                else contextlib.nullcontext()
            )
            with loop_ctx:
              for bi in range(nbatch):
                t0 = t0_pool.tile([P, F], f32)
                for ci in range(BATCH):
                    c = bi * BATCH + ci
                    h_sb = h_pool.tile([P, JT * D], f32)
                    nc.sync.dma_start(out=h_sb[:, :], in_=hv[c])
                    if ci % 2 == 0:
                        # one PSUM bank holds two chunks' logits (2*192=384)
                        psum_g = g_psum_pool.tile([P, 2 * FD], f32)
                    goff = (ci % 2) * FD
                    for half in range(2):
                        psum_hT = hT_psum_pool.tile([P, 1024], f32)
                        for b in range(8):
                            m = 8 * half + b
                            nc.tensor.transpose(
                                psum_hT[:, 128 * b : 128 * (b + 1)],
                                h_sb[:, 128 * m : 128 * (m + 1)],
                                ident_sb[:, :],
                            )
                        sb_hT = hT_sb_pool.tile([P, 1024], f32)
                        nc.scalar.activation(
                            sb_hT[:, :], psum_hT[:, :], Act.Copy
                        )
                        for b in range(8):
                            m = 8 * half + b
                            nc.tensor.matmul(
                                psum_g[:, goff + 12 * m : goff + 12 * (m + 1)],
                                sb_hT[:, 128 * b : 128 * (b + 1)],
                                wg_sb[:, :],
                                start=True,
                                stop=True,
                            )
                    if warm and warm_n:
                        nc.tensor.matmul(
                            warm_ps[:, 0:warm_n],
                            dummy_w[:, :],
                            dummy_x[:, 0:warm_n],
                            start=True,
                            stop=True,
                        )
                    if ci % 2 == 1:
                        # bias add (DVE): psum pair -> batch tile section
                        in4 = psum_g[:, :].rearrange(
                            "c (u j x) -> c u j x", u=2, j=JT
                        )
                        out4 = t0[
                            :, (ci - 1) * FD : (ci + 1) * FD
                        ].rearrange("c (u j x) -> c u j x", u=2, j=JT)
                        bias4 = (
                            bias_sb[:, :]
                            .unsqueeze(1)
                            .unsqueeze(1)
                            .broadcast_to([P, 2, JT, E])
                        )
                        nc.vector.tensor_tensor(
                            out=out4, in0=in4, in1=bias4, op=Alu.add
                        )

                # ---- batched top-2 over 4 chunks (16384 tokens) ----
                t3 = t0[:, :].rearrange("c (t x) -> c t x", x=E)
                m1 = work.tile([P, TB], f32)
                nc.vector.tensor_reduce(
                    out=m1[:, :], in_=t3, axis=mybir.AxisListType.X, op=Alu.max
                )
                m1b = m1[:, :].unsqueeze(2).broadcast_to([P, TB, E])
                c1 = work.tile([P, F], f32)
                c13 = c1[:, :].rearrange("c (t x) -> c t x", x=E)
                nc.vector.tensor_tensor(out=c13, in0=t3, in1=m1b, op=Alu.is_ge)
                gm = work.tile([P, F], f32)
                gm3 = gm[:, :].rearrange("c (t x) -> c t x", x=E)
                nc.vector.scalar_tensor_tensor(
                    out=gm3, in0=c13, scalar=-1e30, in1=t3,
                    op0=Alu.mult, op1=Alu.add,
                )
                m2 = work.tile([P, TB], f32)
                nc.vector.tensor_reduce(
                    out=m2[:, :], in_=gm3, axis=mybir.AxisListType.X, op=Alu.max
                )
                m2b = m2[:, :].unsqueeze(2).broadcast_to([P, TB, E])
                mask2 = work.tile([P, F], f32)
                mask23 = mask2[:, :].rearrange("c (t x) -> c t x", x=E)
                nc.vector.tensor_tensor(out=mask23, in0=t3, in1=m2b, op=Alu.is_ge)
                s12 = work.tile([P, TB], f32)
                nc.vector.tensor_tensor(
                    out=s12[:, :], in0=m1[:, :], in1=m2[:, :], op=Alu.add
                )
                s12b = s12[:, :].unsqueeze(2).broadcast_to([P, TB, E])
                dd = work.tile([P, F], f32)
                dd3 = dd[:, :].rearrange("c (t x) -> c t x", x=E)
                nc.vector.scalar_tensor_tensor(
                    out=dd3, in0=t3, scalar=2.0, in1=s12b,
                    op0=Alu.mult, op1=Alu.subtract,
                )
                qq = work.tile([P, F], f32)
                nc.scalar.activation(qq[:, :], dd[:, :], Act.Sigmoid)
                res = work.tile([P, F], f32)
                nc.vector.tensor_tensor(
                    out=res[:, :], in0=mask2[:, :], in1=qq[:, :], op=Alu.mult
                )
                nc.gpsimd.dma_start(
                    out=ov[bi],
                    in_=res[:, :].rearrange("c (k q) -> c k q", k=BATCH),
                )

            if warm:
                warm_sink = const_pool.tile([P, 1], f32)
                nc.scalar.activation(
                    warm_sink[:, :], warm_ps[:, 0:1], Act.Copy
                )
                warm_dram = nc.dram_tensor("warm_sink_d", [P, 1], f32)
                nc.sync.dma_start(out=warm_dram[:, :], in_=warm_sink[:, :])

    nc.finalize()
    return nc


def _aux_inputs(W, b):
    ident = np.eye(P, dtype=np.float32)
    # wglue[64*bb + e, 6*bb' + x] = W[x, e] iff bb == bb'
    wglue = np.zeros((P, 2 * E), dtype=np.float32)
    wglue[0:D, 0:E] = W.T.astype(np.float32)
    wglue[D : 2 * D, E : 2 * E] = W.T.astype(np.float32)
    biasr = np.tile(b.astype(np.float32)[None, :], (P, 1))
    return ident, wglue, biasr


_NC_CACHE = {}


def _get_nc(nsh, repeats=1, warm=False, warm_n=WARM_N):
    key = (nsh, repeats, warm, warm_n)
    if key not in _NC_CACHE:
        _NC_CACHE[key] = _build_nc(nsh, repeats, warm, warm_n)
    return _NC_CACHE[key]


def kernel(h, W, b):
    global LAST_RESULTS
    from concourse.bass_utils import run_bass_kernel_spmd

    h = np.ascontiguousarray(np.asarray(h, dtype=np.float32))
    W = np.asarray(W, dtype=np.float32)
    b = np.asarray(b, dtype=np.float32)
    n = h.shape[0]
    nsh = n // NCORES
    nc = _get_nc(nsh, warm=True, warm_n=WARM_N)
    ident, wglue, biasr = _aux_inputs(W, b)
    in_maps = []
    for i in range(NCORES):
        in_maps.append(
            {
                "h": h[i * nsh : (i + 1) * nsh],
                "ident": ident,
                "wglue": wglue,
                "biasr": biasr,
            }
        )
    trace = bool(int(os.environ.get("KERNEL_TRACE", "0")))
    res = run_bass_kernel_spmd(
        nc, in_maps, list(range(NCORES)), trace=trace
    )
    LAST_RESULTS = res
    outs = [res.results[i]["out"] for i in range(NCORES)]
    return np.concatenate(outs, axis=0)


# revision 9
# speedup vs baseline: 2.3360x; 2.3360x over previous
"""Trainium2 Bass kernel for FeatureOnlyGate MoE routing (v2).

Math: g = h @ W.T + b  (h: [N,64], W: [6,64], b: [6])
      out[e] = [g_e >= m2] * sigmoid(2*g_e - m1 - m2)
      where m1/m2 are the top-2 logits per token (full-softmax denominator
      cancels after top-2 masking; for the top-1 entry 2g-m1-m2 = m1-m2,
      for the top-2 entry it's m2-m1, others are masked to 0).

All logit math is full fp32: min gap between 2nd and 3rd logit on this
data is 2.5e-7, so reduced-precision matmuls flip top-2 selections.

Changes vs the original baseline (302us):
  - JT=32 tokens/partition (8KB DMA lines in, 768B contiguous runs out:
    both >= 512B so no half-bandwidth DMA penalty on either direction),
    4x fewer DMA instructions, h_pool 6 deep for DMA prefetch.
  - PSUM->SBUF hT copies on ACT as 2x [128,1024] (2-bank PSUM tiles);
    logits of 2 chunks packed per PSUM bank with one fused DVE bias-add;
    g_psum 3 banks deep so the per-batch DVE top-2 burst never
    back-pressures the matmul->bias drain into the DMA stream.
  - top-2 trimmed to 8 DVE ops/batch with the sigmoid on ACT; output DMA
    issued from the Pool SWDGE queue so its semaphore wait (end of the
    DVE chain) cannot head-of-line block h-load issue on the SP queue.
  - PE work per 4096-token chunk: 16 fp32 pair-transposes + 16 12-col
    fp32 matmuls; 12 warm matmuls at start ramp the PE p-state.

Per-core dataflow (data parallel over 8 cores, 262144 tokens each):
  DMA h chunks [128 part, 2048] (32 token rows = 8KB contiguous per
  partition) -> 16x PE pair-transpose [128,128] into 4 PSUM banks ->
  ACT/Pool copy PSUM->SBUF -> 16 fp32 matmuls (stationary hT block,
  moving block-diagonal glued gate weight [128,12]) -> logits land
  token-major [128, 192] in PSUM -> Pool bias-add into batch tile ->
  batched DVE top-2 + ACT sigmoid -> contiguous DMA out.
"""

import os
import numpy as np

N_FULL = 2097152
D = 64
E = 6
NCORES = 8
NSH = N_FULL // NCORES  # 262144 tokens per core

P = 128         # partitions
JT = 32         # consecutive tokens per partition per chunk
CHUNK = P * JT  # 4096 tokens
FD = JT * E     # 192 logit elems per partition per chunk
BATCH = 4       # chunks per top-2 batch
TOKB = CHUNK * BATCH  # 16384 tokens per batch
F = BATCH * FD        # 768 logit elems per partition per batch
TB = BATCH * JT       # 128 tokens per partition per batch

WARM_N = 256    # per-chunk PE keep-hot matmul width (bf16 cols); 0=off

LAST_RESULTS = None  # BassKernelResults of the last hardware run (for test.py)


def _build_nc(nsh, repeats=1, warm=False, warm_n=WARM_N):
    import concourse.bass as bass
    from concourse import bacc
    import concourse.mybir as mybir
    from concourse.tile import TileContext

    f32 = mybir.dt.float32
    Alu = mybir.AluOpType
    Act = mybir.ActivationFunctionType

    nbatch = nsh // TOKB
    assert nsh % TOKB == 0

    nc = bacc.Bacc(None)
    h = nc.declare_dram_parameter("h", [nsh, D], f32, isOutput=False)
    ident = nc.declare_dram_parameter("ident", [P, P], f32, isOutput=False)
    wglue = nc.declare_dram_parameter("wglue", [P, 2 * E], f32, isOutput=False)
    biasr = nc.declare_dram_parameter("biasr", [P, E], f32, isOutput=False)
    out = nc.declare_dram_parameter("out", [nsh, E], f32, isOutput=True)

    # token n = chunk*4096 + 32*p + j  ->  per chunk: [128 part, 2048] with
    # 32 full token rows (8KB) contiguous per partition.
    hv = h[:, :].rearrange("(nc p j) e -> nc p (j e)", p=P, j=JT)
    # out per batch: [128 part, 4 chunk, 192]; (j x) = 768B contiguous in DRAM.
    ov = out[:, :].rearrange(
        "(nb c p j) x -> nb p c (j x)", c=BATCH, p=P, j=JT
    )

    with TileContext(nc) as tc:
        with (
            tc.tile_pool(name="const_pool", bufs=1) as const_pool,
            tc.tile_pool(name="h_pool", bufs=6) as h_pool,
            tc.tile_pool(name="hT_psum", bufs=2, space="PSUM") as hT_psum_pool,
            tc.tile_pool(name="hT_sb", bufs=3) as hT_sb_pool,
            tc.tile_pool(name="g_psum", bufs=3, space="PSUM") as g_psum_pool,
            tc.tile_pool(name="t0_pool", bufs=2) as t0_pool,
            tc.tile_pool(name="work", bufs=2) as work,
        ):
            ident_sb = const_pool.tile([P, P], f32)
            wg_sb = const_pool.tile([P, 2 * E], f32)
            bias_sb = const_pool.tile([P, E], f32)
            nc.sync.dma_start(out=ident_sb[:, :], in_=ident[:, :])
            nc.sync.dma_start(out=wg_sb[:, :], in_=wglue[:, :])
            nc.sync.dma_start(out=bias_sb[:, :], in_=biasr[:, :])

            if warm:
                bf16 = mybir.dt.bfloat16
                dummy_w = const_pool.tile([P, P], bf16)
                dummy_x = const_pool.tile([P, 512], bf16)
                nc.vector.memset(dummy_w[:, :], 0.0)
                nc.vector.memset(dummy_x[:, :], 0.0)

                warm_ps = g_psum_pool.tile([P, 512], f32, tag="warm", bufs=1)
                for _ in range(12):
                    nc.tensor.matmul(
                        warm_ps[:, :], dummy_w[:, :], dummy_x[:, :],
                        start=True, stop=True,
                    )

            import contextlib

            loop_ctx = (
                tc.For_i(0, repeats, 1)
                if repeats > 1
                else contextlib.nullcontext()
            )
            with loop_ctx:
              for bi in range(nbatch):
                t0 = t0_pool.tile([P, F], f32)
                for ci in range(BATCH):
                    c = bi * BATCH + ci
                    h_sb = h_pool.tile([P, JT * D], f32)
                    nc.sync.dma_start(out=h_sb[:, :], in_=hv[c])
                    if ci % 2 == 0:
                        # one PSUM bank holds two chunks' logits (2*192=384)
                        psum_g = g_psum_pool.tile([P, 2 * FD], f32)
                    goff = (ci % 2) * FD
                    for half in range(2):
                        psum_hT = hT_psum_pool.tile([P, 1024], f32)
                        for b in range(8):
                            m = 8 * half + b
                            nc.tensor.transpose(
                                psum_hT[:, 128 * b : 128 * (b + 1)],
                                h_sb[:, 128 * m : 128 * (m + 1)],
                                ident_sb[:, :],
                            )
                        sb_hT = hT_sb_pool.tile([P, 1024], f32)
                        nc.scalar.activation(
                            sb_hT[:, :], psum_hT[:, :], Act.Copy
                        )
                        for b in range(8):
                            m = 8 * half + b
                            nc.tensor.matmul(
                                psum_g[:, goff + 12 * m : goff + 12 * (m + 1)],
                                sb_hT[:, 128 * b : 128 * (b + 1)],
                                wg_sb[:, :],
                                start=True,
                                stop=True,
                            )
                    if warm and warm_n:
                        nc.tensor.matmul(
                            warm_ps[:, 0:warm_n],
                            dummy_w[:, :],
                            dummy_x[:, 0:warm_n],
                            start=True,
                            stop=True,
                        )
                    if ci % 2 == 1:
                        # bias add (DVE): psum pair -> batch tile section
                        in4 = psum_g[:, :].rearrange(
                            "c (u j x) -> c u j x", u=2, j=JT
                        )
                        out4 = t0[
                            :, (ci - 1) * FD : (ci + 1) * FD
                        ].rearrange("c (u j x) -> c u j x", u=2, j=JT)
                        bias4 = (
                            bias_sb[:, :]
                            .unsqueeze(1)
                            .unsqueeze(1)
                            .broadcast_to([P, 2, JT, E])
                        )
                        nc.vector.tensor_tensor(
                            out=out4, in0=in4, in1=bias4, op=Alu.add
                        )

                # ---- batched top-2 over 4 chunks (16384 tokens) ----
                t3 = t0[:, :].rearrange("c (t x) -> c t x", x=E)
                m1 = work.tile([P, TB], f32)
                nc.vector.tensor_reduce(
                    out=m1[:, :], in_=t3, axis=mybir.AxisListType.X, op=Alu.max
                )
                m1b = m1[:, :].unsqueeze(2).broadcast_to([P, TB, E])
                c1 = work.tile([P, F], f32)
                c13 = c1[:, :].rearrange("c (t x) -> c t x", x=E)
                nc.vector.tensor_tensor(out=c13, in0=t3, in1=m1b, op=Alu.is_ge)
                gm = work.tile([P, F], f32)
                gm3 = gm[:, :].rearrange("c (t x) -> c t x", x=E)
                nc.vector.scalar_tensor_tensor(
                    out=gm3, in0=c13, scalar=-1e30, in1=t3,
                    op0=Alu.mult, op1=Alu.add,
                )
                m2 = work.tile([P, TB], f32)
                nc.vector.tensor_reduce(
                    out=m2[:, :], in_=gm3, axis=mybir.AxisListType.X, op=Alu.max
                )
                m2b = m2[:, :].unsqueeze(2).broadcast_to([P, TB, E])
                mask2 = work.tile([P, F], f32)
                mask23 = mask2[:, :].rearrange("c (t x) -> c t x", x=E)
                nc.vector.tensor_tensor(out=mask23, in0=t3, in1=m2b, op=Alu.is_ge)
                s12 = work.tile([P, TB], f32)
                nc.vector.tensor_tensor(
                    out=s12[:, :], in0=m1[:, :], in1=m2[:, :], op=Alu.add
                )
                s12b = s12[:, :].unsqueeze(2).broadcast_to([P, TB, E])
                dd = work.tile([P, F], f32)
                dd3 = dd[:, :].rearrange("c (t x) -> c t x", x=E)
                nc.vector.scalar_tensor_tensor(
                    out=dd3, in0=t3, scalar=2.0, in1=s12b,
                    op0=Alu.mult, op1=Alu.subtract,
                )
                qq = work.tile([P, F], f32)
                nc.scalar.activation(qq[:, :], dd[:, :], Act.Sigmoid)
                res = work.tile([P, F], f32)
                nc.vector.tensor_tensor(
                    out=res[:, :], in0=mask2[:, :], in1=qq[:, :], op=Alu.mult
                )
                nc.gpsimd.dma_start(
                    out=ov[bi],
                    in_=res[:, :].rearrange("c (k q) -> c k q", k=BATCH),
                )

            if warm:
                warm_sink = const_pool.tile([P, 1], f32)
                nc.scalar.activation(
                    warm_sink[:, :], warm_ps[:, 0:1], Act.Copy
                )
                warm_dram = nc.dram_tensor("warm_sink_d", [P, 1], f32)
                nc.sync.dma_start(out=warm_dram[:, :], in_=warm_sink[:, :])

    nc.finalize()
    return nc


def _aux_inputs(W, b):
    ident = np.eye(P, dtype=np.float32)
    # wglue[64*bb + e, 6*bb' + x] = W[x, e] iff bb == bb'
    wglue = np.zeros((P, 2 * E), dtype=np.float32)
    wglue[0:D, 0:E] = W.T.astype(np.float32)
    wglue[D : 2 * D, E : 2 * E] = W.T.astype(np.float32)
    biasr = np.tile(b.astype(np.float32)[None, :], (P, 1))
    return ident, wglue, biasr


_NC_CACHE = {}


def _get_nc(nsh, repeats=1, warm=False, warm_n=WARM_N):
    key = (nsh, repeats, warm, warm_n)
    if key not in _NC_CACHE:
        _NC_CACHE[key] = _build_nc(nsh, repeats, warm, warm_n)
    return _NC_CACHE[key]


def kernel(h, W, b):
    global LAST_RESULTS
    from concourse.bass_utils import run_bass_kernel_spmd

    h = np.ascontiguousarray(np.asarray(h, dtype=np.float32))
    W = np.asarray(W, dtype=np.float32)
    b = np.asarray(b, dtype=np.float32)
    n = h.shape[0]
    nsh = n // NCORES
    nc = _get_nc(nsh, warm=True, warm_n=WARM_N)
    ident, wglue, biasr = _aux_inputs(W, b)
    in_maps = []
    for i in range(NCORES):
        in_maps.append(
            {
                "h": h[i * nsh : (i + 1) * nsh],
                "ident": ident,
                "wglue": wglue,
                "biasr": biasr,
            }
        )
    trace = bool(int(os.environ.get("KERNEL_TRACE", "0")))
    res = run_bass_kernel_spmd(
        nc, in_maps, list(range(NCORES)), trace=trace
    )
    LAST_RESULTS = res
    outs = [res.results[i]["out"] for i in range(NCORES)]
    return np.concatenate(outs, axis=0)
